# revision 35
# baseline (speedup 1.0000x reference)
"""Trainium2 Bass kernel for nn_ActorPPO (stock-news actor network).

Sharding: one stock per NeuronCore (S=8, n_cores=8). Each core runs the
full per-stock pipeline on device: 3 conv1d+maxpool branches (bf16
matmuls, fp32 PSUM accumulate), 10-step LSTM + degenerate attention
(h-sum), linX/linY MLP, and its stock's slice of the final arch MLP's
first layer. Host-side gather = sum of per-core partials + relu + the
tiny [8,256] second layer + tanh.
"""

import sys
import types

sys.path.insert(0, "/opt/trn_rl_repo")

import numpy as np
import ml_dtypes


def _ensure_ntff_hook():
    """Register the axon NTFF profiling hook if the image's antenv lacks it."""
    try:
        from antenv.axon_hooks import get_axon_ntff_profile_hook  # noqa: F401
        return
    except ImportError:
        pass
    try:
        import antenv
        from trn_agent_boot.trn_boot import _ntff_profile_via_ctypes
    except ImportError:
        return
    mod = types.ModuleType("antenv.axon_hooks")
    _hook = {"h": None}
    mod.set_axon_ntff_profile_hook = lambda h: _hook.__setitem__("h", h)
    mod.get_axon_ntff_profile_hook = lambda: _hook["h"]
    sys.modules["antenv.axon_hooks"] = mod
    antenv.axon_hooks = mod
    mod.set_axon_ntff_profile_hook(
        _ntff_profile_via_ctypes("/opt/axon/libaxon_pjrt.so"))


_ensure_ntff_hook()

from concourse import bacc  # noqa: E402
import concourse.bass as bass  # noqa: E402
import concourse.mybir as mybir  # noqa: E402
import concourse.tile as tile  # noqa: E402
from concourse.bass_utils import run_bass_kernel_spmd  # noqa: E402

F32 = mybir.dt.float32
BF16 = mybir.dt.bfloat16
AFT = mybir.ActivationFunctionType
AX = mybir.AxisListType
AluOp = mybir.AluOpType
FP8 = mybir.dt.float8e4
DROW = mybir.MatmulPerfMode.DoubleRow

BS, S, D, N, E = 16, 8, 10, 32, 300
H, STATE_DIM, MID, OC = 128, 96, 256, 100
NCORES = 8
KS = (3, 4, 5)
BF = ml_dtypes.bfloat16
F8 = ml_dtypes.float8_e4m3

# PyTorch gate order in weights is (i, f, g, o); we re-order rows to
# (i, f, o, g) so one sigmoid covers cols 0:48 and one tanh cols 48:64.
_GPERM = np.concatenate([
    np.arange(0, 128), np.arange(128, 256),
    np.arange(384, 512), np.arange(256, 384)])

_cache = {}


def _build(debug_taps=False):
    nc = bacc.Bacc("TRN2", target_bir_lowering=False, debug=False,
                   enable_asserts=True, num_devices=NCORES)

    news8 = nc.dram_tensor("news8", [128, 2, D * BS * N], FP8,
                           kind="ExternalInput")
    news2 = nc.dram_tensor("news2", [44, D * BS * N], BF16,
                           kind="ExternalInput")
    cw8 = {k: nc.dram_tensor(f"cw8{k}", [128, 2, k, 112], FP8,
                             kind="ExternalInput") for k in KS}
    cws = nc.dram_tensor("cws", [88, 2, OC], BF16, kind="ExternalInput")
    cwm = nc.dram_tensor("cwm", [88, 5, OC], BF16, kind="ExternalInput")
    wih = nc.dram_tensor("wih", [101, 3, 4 * H], BF16, kind="ExternalInput")
    whh = nc.dram_tensor("whh", [H, 4 * H], BF16, kind="ExternalInput")
    bias = nc.dram_tensor("bias", [128, 71], F32, kind="ExternalInput")
    lxw = nc.dram_tensor("lxw", [128, 2, H], BF16, kind="ExternalInput")
    lyw = nc.dram_tensor("lyw", [H, 64], BF16, kind="ExternalInput")
    sft = nc.dram_tensor("sft", [STATE_DIM, BS], BF16, kind="ExternalInput")
    w1t = nc.dram_tensor("w1t", [STATE_DIM, 24], BF16, kind="ExternalInput")
    w2t = nc.dram_tensor("w2t", [24, 16], BF16, kind="ExternalInput")
    aw1s = nc.dram_tensor("aw1s", [64, MID], BF16, kind="ExternalInput")
    aw1f = nc.dram_tensor("aw1f", [16, MID], BF16, kind="ExternalInput")
    outq = nc.dram_tensor("outq", [128, 2, BS], F32, kind="ExternalOutput")
    if debug_taps:
        dbg_text = nc.dram_tensor("dbg_text", [3, OC, D * BS], F32,
                                  kind="ExternalOutput")
        dbg_h = nc.dram_tensor("dbg_h", [H, BS], F32, kind="ExternalOutput")
        dbg_hsum = nc.dram_tensor("dbg_hsum", [H, BS], F32,
                                  kind="ExternalOutput")
        dbg_hy = nc.dram_tensor("dbg_hy", [64, BS], F32, kind="ExternalOutput")
        dbg_f2 = nc.dram_tensor("dbg_f2", [16, BS], F32, kind="ExternalOutput")

    with tile.TileContext(nc) as tc:
        with (
            tc.tile_pool(name="xp", bufs=1) as xp,
            tc.tile_pool(name="wp", bufs=1) as wp,
            tc.tile_pool(name="st", bufs=1) as st,
        ):
            # ---- weight / input DMAs ----
            # day-block split so conv can start before the full news lands
            DBLK = ((0, 3), (3, 5), (5, 8), (8, 10))
            # x8: e-pairs 0:256 as fp8 [ki, s, pos] for DoubleRow passes
            x8 = xp.tile([128, 2, D * BS * N], FP8, tag="x8")
            # xm01 interleaved: partition 2i = e-chunk2 row i, partition
            # 2i+1 = same row shifted one news position (dense K=88
            # tap-pair packing, full-width DMA via overlapping source AP)
            xm01 = xp.tile([88, D * BS * N], BF16, tag="xm01")
            dum = xp.tile([128, 512], BF16, tag="dum")
            nc.vector.memset(dum[:], 0.0)
            cw_sb = {k: wp.tile([128, 2, k, 112], FP8, tag=f"cw{k}",
                                name=f"cwsb{k}") for k in KS}
            cws_sb = wp.tile([88, 2, OC], BF16, tag="cws")
            cwm_sb = wp.tile([88, 5, OC], BF16, tag="cwm")
            wih_sb = wp.tile([101, 3, 4 * H], BF16, tag="wih")
            whh_sb = wp.tile([H, 4 * H], BF16, tag="whh")
            bias_sb = wp.tile([128, 71], F32, tag="bias")
            lxw_sb = wp.tile([128, 2, H], BF16, tag="lxw")
            lyw_sb = wp.tile([H, 64], BF16, tag="lyw")
            sft_sb = wp.tile([STATE_DIM, BS], BF16, tag="sft")
            w1t_sb = wp.tile([STATE_DIM, 24], BF16, tag="w1t")
            w2t_sb = wp.tile([24, 16], BF16, tag="w2t")
            aw1s_sb = wp.tile([64, MID], BF16, tag="aw1s")
            aw1f_sb = wp.tile([16, MID], BF16, tag="aw1f")

            # two fast HWDGE rings (sync / scalar), deadline-ordered;
            # gpsimd (slow SWDGE) only carries tiny head weights
            nc.scalar.dma_start(bias_sb[:], bias[:])
            nc.scalar.dma_start(sft_sb[:], sft[:])
            nc.scalar.dma_start(w1t_sb[:], w1t[:])
            nc.scalar.dma_start(w2t_sb[:], w2t[:])

            n2t = news2[:].tensor

            def news_block(bi):
                lo, hi = DBLK[bi]
                a, b = lo * BS * N, hi * BS * N
                nc.sync.dma_start(x8[:, :, a:b], news8[:, :, a:b])
                blk = b - a - (1 if hi == D else 0)
                m_src = bass.AP(tensor=n2t, offset=a,
                                ap=[[D * BS * N, 44], [1, 2], [1, blk]])
                nc.scalar.dma_start(xm01[0:88, a:a + blk], m_src)

            news_block(0)
            nc.sync.dma_start(cw_sb[3][:], cw8[3][:])
            nc.scalar.dma_start(cw_sb[4][:], cw8[4][:])
            nc.scalar.dma_start(cw_sb[5][:], cw8[5][:])
            nc.scalar.dma_start(cwm_sb[:], cwm[:])
            nc.scalar.dma_start(cws_sb[:], cws[:])
            news_block(1)
            nc.sync.dma_start(wih_sb[:], wih[:])
            nc.scalar.dma_start(whh_sb[:], whh[:])
            news_block(2)
            news_block(3)
            nc.sync.dma_start(lxw_sb[:], lxw[:])
            nc.scalar.dma_start(lyw_sb[:], lyw[:])
            nc.sync.dma_start(aw1s_sb[:], aw1s[:])
            nc.scalar.dma_start(aw1f_sb[:], aw1f[:])

            # views of news: [p, (s,) day, batch, news-pos]
            x8v = x8[:].rearrange("p s (d b n) -> p s d b n", d=D, b=BS)
            xmv = xm01[:].rearrange("p (d b n) -> p d b n", d=D, b=BS)

            # persistent state
            textk = [st.tile([OC + 1 if c == 0 else OC, D * BS], BF16,
                             tag=f"text{c}", name=f"text{c}")
                     for c in range(3)]
            nc.vector.memset(textk[0][:], 1.0)
            h_sb = st.tile([H, BS], F32, tag="h")
            c_sb = st.tile([H, BS], F32, tag="c")
            hsum = st.tile([H, BS], F32, tag="hsum")
            nc.vector.memset(h_sb[:], 0.0)
            nc.vector.memset(c_sb[:], 0.0)
            tanc = st.tile([H, BS], F32, tag="tanc")
            tmp = st.tile([H, BS], F32, tag="tmp")
            h_bf = st.tile([H, BS], BF16, tag="h_bf")
            hsum_bf = st.tile([H, BS], BF16, tag="hsum_bf")
            nc.vector.memset(h_bf[:], 0.0)

            with (
                tc.tile_pool(name="psA", bufs=2, space="PSUM") as psA,
                tc.tile_pool(name="psPre", bufs=2, space="PSUM") as psPre,
                tc.tile_pool(name="gp", bufs=3) as gp,
            ):
                # PE warmup during the news DMA window
                for r in range(9):
                    wps = psA.tile([OC, 512], F32, tag="conv5",
                                   name=f"warm{r}")
                    nc.tensor.matmul(wps[:], dum[:, 0:OC], dum[:],
                                     start=True, stop=True)
                # feats branch rides the warmup window (tiny, independent)
                fp1 = psA.tile([24, BS], F32, tag="conv4", name="fp1")
                nc.tensor.matmul(fp1[:], w1t_sb[:], sft_sb[:],
                                 start=True, stop=True)
                f1 = st.tile([24, BS], BF16, tag="f1")
                nc.scalar.activation(f1[:], fp1[:], AFT.Relu,
                                     bias=bias_sb[0:24, 68:69])
                fp2 = psA.tile([16, BS], F32, tag="conv4", name="fp2")
                nc.tensor.matmul(fp2[:], w2t_sb[:], f1[:],
                                 start=True, stop=True)
                f2 = st.tile([16, BS], BF16, tag="f2")
                nc.scalar.activation(f2[:], fp2[:], AFT.Identity,
                                     bias=bias_sb[0:16, 69:70])

                def conv_day(d):
                    for ki_, k in enumerate(KS):
                        L = N - k + 1
                        cp = psA.tile([112, BS, L], F32, tag=f"conv{k}",
                                      name=f"cp{k}_{d}")
                        # fp8 DoubleRow passes: 256 taps (e 0:256) per j
                        for j in range(k):
                            nc.tensor.matmul(
                                cp[:], cw_sb[k][:, :, j, :],
                                x8v[:, :, d, :, j:j + L],
                                start=(j == 0), stop=False,
                                skip_group_check=True, perf_mode=DROW)
                        # bf16 remainder (e 256:300): dense 88-row merged
                        # tap-pairs + leftover 44-row taps
                        passes = []
                        mslot = {3: [0], 4: [1, 2], 5: [3, 4]}[k]
                        for pi, mi in enumerate(mslot):
                            j = 2 * pi
                            passes.append((cwm_sb[:, mi, :],
                                           xmv[:, d, :, j:j + L]))
                        if k in (3, 5):
                            si_ = 0 if k == 3 else 1
                            j = k - 1
                            passes.append((cws_sb[:, si_, :],
                                           xmv[:, d, :, j:j + L]))
                        for i, (lh, rh) in enumerate(passes):
                            nc.tensor.matmul(cp[0:OC, :, :], lh, rh,
                                             start=False,
                                             stop=(i == len(passes) - 1),
                                             skip_group_check=True)
                        nc.vector.reduce_max(
                            textk[ki_][0:OC, d * BS:(d + 1) * BS],
                            cp[0:OC, :, :], axis=AX.X)

                def pre_pair(dp):
                    # NOTE: start=True clears has_written for the whole PSUM
                    # bank, so only the first matmul touching the bank sets
                    # it; later matmuls rely on per-element has_written bits.
                    pre_t = psPre.tile([128, 2, 4, BS], F32, tag="pre",
                                       name=f"pre{dp}")
                    for g in range(4):
                        for c in range(3):
                            kc = OC + 1 if c == 0 else OC
                            nc.tensor.matmul(
                                pre_t[:, :, g, :],
                                wih_sb[0:kc, c, g * H:(g + 1) * H],
                                textk[c][:, dp * 2 * BS:(dp + 1) * 2 * BS],
                                start=(g == 0 and c == 0), stop=False,
                                skip_group_check=True)
                    return pre_t

                def lstm_step(d, pre_t):
                    dsub = d % 2
                    for g in range(4):
                        nc.tensor.matmul(
                            pre_t[:, dsub, g, :],
                            whh_sb[:, g * H:(g + 1) * H],
                            h_bf[:],
                            start=False, stop=True,
                            skip_group_check=True)
                    # gate bias rides the K=101 ones-row; tanh(x) for the
                    # g-gate is computed as 2*sigmoid(2x)-1 (weights 2x'd on
                    # host) so one table lookup covers all four gates.
                    act = gp.tile([128, 4, BS], F32, tag="act",
                                  name=f"act{d}")
                    nc.scalar.activation(act[:], pre_t[:, dsub, :, :],
                                         AFT.Sigmoid)
                    i_s = act[:, 0, :]
                    f_s = act[:, 1, :]
                    o_s = act[:, 2, :]
                    g2 = gp.tile([128, BS], F32, tag="g2", name=f"g2_{d}")
                    nc.vector.tensor_scalar(g2[:], act[:, 3, :], 2.0, -1.0,
                                            AluOp.mult, AluOp.add)
                    nc.vector.tensor_mul(c_sb[:], c_sb[:], f_s)
                    nc.vector.tensor_mul(tmp[:], i_s, g2[:])
                    nc.vector.tensor_add(c_sb[:], c_sb[:], tmp[:])
                    nc.scalar.activation(tanc[:], c_sb[:], AFT.Tanh)
                    nc.vector.tensor_mul(h_sb[:], o_s, tanc[:])
                    nc.vector.tensor_mul(h_bf[:], o_s, tanc[:])
                    if d == 0:
                        nc.vector.tensor_copy(hsum[:], h_sb[:])
                    else:
                        nc.vector.tensor_add(hsum[:], hsum[:], h_sb[:])

                # software pipeline: gates lag conv by one day-pair,
                # interleaved between conv days so the PE never stalls on
                # the LSTM recurrence chain
                pre_tiles = {}
                for dp in range(5):
                    conv_day(2 * dp)
                    if dp > 0:
                        lstm_step(2 * dp - 2, pre_tiles[dp - 1])
                    conv_day(2 * dp + 1)
                    if dp > 0:
                        lstm_step(2 * dp - 1, pre_tiles[dp - 1])
                    pre_tiles[dp] = pre_pair(dp)
                lstm_step(8, pre_tiles[4])
                lstm_step(9, pre_tiles[4])

            # ---- head: linX, linY, feats branch, arch partial ----
            with (
                tc.tile_pool(name="ps2", bufs=1, space="PSUM") as ps2,
                tc.tile_pool(name="fin", bufs=1) as fin,
            ):
                nc.vector.tensor_copy(hsum_bf[:], hsum[:])
                px = ps2.tile([H, BS], F32, tag="px")
                nc.tensor.matmul(px[:], lxw_sb[:, 0, :], h_bf[:],
                                 start=True, stop=False)
                nc.tensor.matmul(px[:], lxw_sb[:, 1, :], hsum_bf[:],
                                 start=False, stop=True)
                hx = fin.tile([H, BS], BF16, tag="hx")
                nc.scalar.activation(hx[:], px[:], AFT.Relu,
                                     bias=bias_sb[:, 64:65])
                py = ps2.tile([64, BS], F32, tag="py")
                nc.tensor.matmul(py[:], lyw_sb[:], hx[:],
                                 start=True, stop=True)
                hy = fin.tile([64, BS], BF16, tag="hy")
                nc.scalar.activation(hy[:], py[:], AFT.Relu,
                                     bias=bias_sb[0:64, 65:66])

                qp = ps2.tile([128, 2, BS], F32, tag="qp")
                for m in range(2):
                    nc.tensor.matmul(qp[:, m, :],
                                     aw1s_sb[:, m * 128:(m + 1) * 128],
                                     hy[:], start=True, stop=False)
                    nc.tensor.matmul(qp[:, m, :],
                                     aw1f_sb[:, m * 128:(m + 1) * 128],
                                     f2[:], start=False, stop=True)
                qsb = fin.tile([128, 2, BS], F32, tag="qsb")
                for m in range(2):
                    nc.scalar.activation(qsb[:, m, :], qp[:, m, :],
                                         AFT.Identity,
                                         bias=bias_sb[:, 66 + m:67 + m])
                nc.sync.dma_start(outq[:], qsb[:])
                if debug_taps:
                    for c in range(3):
                        nc.sync.dma_start(dbg_text[c], textk[c][0:OC, :])
                    nc.sync.dma_start(dbg_h[:], h_sb[:])
                    nc.sync.dma_start(dbg_hsum[:], hsum[:])
                    nc.sync.dma_start(dbg_hy[:], hy[:])
                    nc.sync.dma_start(dbg_f2[:], f2[:])

    nc.compile()
    return nc


def _prep_inputs(inp):
    """Host-side shard/layout prep. Returns (in_maps, aw2, ab2, sigma)."""
    f32 = np.float32
    news = np.asarray(inp["stock_news"], f32)      # [B,S,D,N,E]
    # -> [S, E, D, B, N]
    newsT = np.ascontiguousarray(news.transpose(1, 4, 2, 0, 3))
    newsT = newsT.reshape(S, E, D * BS * N)

    sf = np.asarray(inp["stock_feats"], f32)
    w1 = np.asarray(inp["w1"], f32)
    w2 = np.asarray(inp["w2"], f32)
    aw1 = np.asarray(inp["arch_w1"], f32)          # [MID, 16+64*S]
    ab1 = np.asarray(inp["arch_b1"], f32)
    aw1f = np.ascontiguousarray((aw1[:, 64 * S:] / 8.0).T).astype(BF)
    sft = np.ascontiguousarray(sf.T).astype(BF)
    w1t = np.ascontiguousarray(w1.T).astype(BF)
    w2t = np.ascontiguousarray(w2.T).astype(BF)

    in_maps = []
    for s in range(S):
        m = {}
        # e 0:256 as fp8 [ki, s2, pos] (DoubleRow), e 256:300 as bf16
        m["news8"] = np.ascontiguousarray(
            newsT[s, 0:256]).reshape(128, 2, D * BS * N).astype(F8)
        m["news2"] = np.ascontiguousarray(newsT[s, 256:300]).astype(BF)
        cbs = []
        cws = np.zeros((88, 2, OC), np.float32)
        for k in KS:
            w = np.asarray(inp[f"conv_w{k}"], np.float32)[s]  # [OC, E, k]
            cbs.append(np.asarray(inp[f"conv_b{k}"], np.float32)[s])
            wt = w.transpose(1, 2, 0)                     # [E, k, OC]
            w8 = np.zeros((128, 2, k, 112), np.float32)
            w8[:, :, :, 0:OC] = wt[0:256].reshape(128, 2, k, OC)
            m[f"cw8{k}"] = w8.astype(F8)
            if k in (3, 5):
                cws[0::2, 0 if k == 3 else 1, :] = wt[256:300, k - 1, :]
        m["cws"] = cws.astype(BF)
        cwm = np.zeros((88, 5, OC), np.float32)
        for idx, (k, j) in enumerate(((3, 0), (4, 0), (4, 2), (5, 0), (5, 2))):
            w = np.asarray(inp[f"conv_w{k}"], np.float32)[s]  # [OC, E, k]
            cwm[0::2, idx, :] = w[:, 256:300, j].T
            cwm[1::2, idx, :] = w[:, 256:300, j + 1].T
        m["cwm"] = cwm.astype(BF)
        cb_full = np.concatenate(cbs)                     # [300]
        w_ih = np.asarray(inp["w_ih"], f32)[s]            # [4H, 300]
        w_hh = np.asarray(inp["w_hh"], f32)[s]            # [4H, H]
        b_ih = np.asarray(inp["b_ih"], f32)[s]
        b_hh = np.asarray(inp["b_hh"], f32)[s]
        # g-gate (cols 384:512 after perm) scaled 2x: tanh(x)=2*sig(2x)-1
        bias_eff = (b_ih + b_hh + w_ih @ cb_full)[_GPERM]  # [512]
        bias_eff[384:512] *= 2.0
        wihp = w_ih[_GPERM].T.copy()                      # [300, 512]
        wihp[:, 384:512] *= 2.0
        wih_arr = np.zeros((101, 3, 4 * H), f32)
        wih_arr[0:100] = wihp.reshape(3, 100, 4 * H).transpose(1, 0, 2)
        wih_arr[100, 0, :] = bias_eff                     # ones-row bias
        m["wih"] = np.ascontiguousarray(wih_arr).astype(BF)
        whhp = w_hh[_GPERM].T.copy()
        whhp[:, 384:512] *= 2.0
        m["whh"] = np.ascontiguousarray(whhp).astype(BF)

        bias = np.zeros((128, 71), f32)
        bias[:, 64] = np.asarray(inp["linX_b"], f32)[s]
        bias[0:64, 65] = np.asarray(inp["linY_b"], f32)[s]
        bias[:, 66:68] = (ab1 / 8.0).reshape(2, 128).T
        bias[0:24, 68] = np.asarray(inp["b1"], f32)
        bias[0:16, 69] = np.asarray(inp["b2"], f32)
        m["bias"] = bias

        lx = np.asarray(inp["linX_w"], f32)[s]            # [H, 2H]
        m["lxw"] = np.ascontiguousarray(
            lx.T.reshape(2, 128, H).transpose(1, 0, 2)).astype(BF)
        m["lyw"] = np.ascontiguousarray(
            np.asarray(inp["linY_w"], f32)[s].T).astype(BF)
        m["sft"] = sft
        m["w1t"] = w1t
        m["w2t"] = w2t
        m["aw1s"] = np.ascontiguousarray(aw1[:, 64 * s:64 * (s + 1)].T).astype(BF)
        m["aw1f"] = aw1f
        in_maps.append(m)

    aw2 = np.asarray(inp["arch_w2"], f32)                 # [S, MID]
    ab2 = np.asarray(inp["arch_b2"], f32)
    sigma = np.exp(np.asarray(inp["action_var"], f32))
    return in_maps, aw2, ab2, sigma


def run(inputs, trace=False, tmpdir=None, debug_taps=False):
    key = ("nc", debug_taps)
    if key not in _cache:
        _cache[key] = _build(debug_taps)
    nc = _cache[key]
    in_maps, aw2, ab2, sigma = _prep_inputs(inputs)
    res = run_bass_kernel_spmd(nc, in_maps, core_ids=list(range(NCORES)),
                               trace=trace, tmpdir=tmpdir)
    # host gather: sum partials -> relu -> layer 2 -> tanh
    A = np.zeros((MID, BS), np.float32)
    for c in range(NCORES):
        q = res.results[c]["outq"]                        # [128, 2, BS]
        A += q.transpose(1, 0, 2).reshape(MID, BS)
    a1 = np.maximum(A, 0.0)
    mu = np.tanh(aw2 @ a1 + ab2[:, None]).T               # [BS, S]
    return (np.ascontiguousarray(mu.astype(np.float32)),
            sigma.astype(np.float32)), res


def kernel(**inputs):
    (mu, sigma), _ = run(inputs, trace=False)
    return mu, sigma


# revision 36
# speedup vs baseline: 1.0150x; 1.0150x over previous
"""Trainium2 Bass kernel for nn_ActorPPO (stock-news actor network).

Sharding: one stock per NeuronCore (S=8, n_cores=8). Each core runs the
full per-stock pipeline on device: 3 conv1d+maxpool branches (bf16
matmuls, fp32 PSUM accumulate), 10-step LSTM + degenerate attention
(h-sum), linX/linY MLP, and its stock's slice of the final arch MLP's
first layer. Host-side gather = sum of per-core partials + relu + the
tiny [8,256] second layer + tanh.
"""

import sys
import types

sys.path.insert(0, "/opt/trn_rl_repo")

import numpy as np
import ml_dtypes


def _ensure_ntff_hook():
    """Register the axon NTFF profiling hook if the image's antenv lacks it."""
    try:
        from antenv.axon_hooks import get_axon_ntff_profile_hook  # noqa: F401
        return
    except ImportError:
        pass
    try:
        import antenv
        from trn_agent_boot.trn_boot import _ntff_profile_via_ctypes
    except ImportError:
        return
    mod = types.ModuleType("antenv.axon_hooks")
    _hook = {"h": None}
    mod.set_axon_ntff_profile_hook = lambda h: _hook.__setitem__("h", h)
    mod.get_axon_ntff_profile_hook = lambda: _hook["h"]
    sys.modules["antenv.axon_hooks"] = mod
    antenv.axon_hooks = mod
    mod.set_axon_ntff_profile_hook(
        _ntff_profile_via_ctypes("/opt/axon/libaxon_pjrt.so"))


_ensure_ntff_hook()

from concourse import bacc  # noqa: E402
import concourse.bass as bass  # noqa: E402
import concourse.mybir as mybir  # noqa: E402
import concourse.tile as tile  # noqa: E402
from concourse.bass_utils import run_bass_kernel_spmd  # noqa: E402

F32 = mybir.dt.float32
BF16 = mybir.dt.bfloat16
AFT = mybir.ActivationFunctionType
AX = mybir.AxisListType
AluOp = mybir.AluOpType
FP8 = mybir.dt.float8e4
DROW = mybir.MatmulPerfMode.DoubleRow

BS, S, D, N, E = 16, 8, 10, 32, 300
H, STATE_DIM, MID, OC = 128, 96, 256, 100
NCORES = 8
KS = (3, 4, 5)
BF = ml_dtypes.bfloat16
F8 = ml_dtypes.float8_e4m3

# PyTorch gate order in weights is (i, f, g, o); we re-order rows to
# (i, f, o, g) so one sigmoid covers cols 0:48 and one tanh cols 48:64.
_GPERM = np.concatenate([
    np.arange(0, 128), np.arange(128, 256),
    np.arange(384, 512), np.arange(256, 384)])

_cache = {}


def _build(debug_taps=False):
    nc = bacc.Bacc("TRN2", target_bir_lowering=False, debug=False,
                   enable_asserts=True, num_devices=NCORES)

    news8 = nc.dram_tensor("news8", [128, 2, D * BS * N], FP8,
                           kind="ExternalInput")
    news2 = nc.dram_tensor("news2", [44, D * BS * N], BF16,
                           kind="ExternalInput")
    cw8 = {k: nc.dram_tensor(f"cw8{k}", [128, 2, k, 112], FP8,
                             kind="ExternalInput") for k in KS}
    cws = nc.dram_tensor("cws", [88, 2, OC], BF16, kind="ExternalInput")
    cwm = nc.dram_tensor("cwm", [88, 5, OC], BF16, kind="ExternalInput")
    wih = nc.dram_tensor("wih", [101, 3, 4 * H], BF16, kind="ExternalInput")
    whh = nc.dram_tensor("whh", [H, 4 * H], BF16, kind="ExternalInput")
    bias = nc.dram_tensor("bias", [128, 71], F32, kind="ExternalInput")
    lxw = nc.dram_tensor("lxw", [128, 2, H], BF16, kind="ExternalInput")
    lyw = nc.dram_tensor("lyw", [H, 64], BF16, kind="ExternalInput")
    sft = nc.dram_tensor("sft", [STATE_DIM, BS], BF16, kind="ExternalInput")
    w1t = nc.dram_tensor("w1t", [STATE_DIM, 24], BF16, kind="ExternalInput")
    w2t = nc.dram_tensor("w2t", [24, 16], BF16, kind="ExternalInput")
    aw1s = nc.dram_tensor("aw1s", [64, MID], BF16, kind="ExternalInput")
    aw1f = nc.dram_tensor("aw1f", [16, MID], BF16, kind="ExternalInput")
    outq = nc.dram_tensor("outq", [128, 2, BS], F32, kind="ExternalOutput")
    if debug_taps:
        dbg_text = nc.dram_tensor("dbg_text", [3, OC, D * BS], F32,
                                  kind="ExternalOutput")
        dbg_h = nc.dram_tensor("dbg_h", [H, BS], F32, kind="ExternalOutput")
        dbg_hsum = nc.dram_tensor("dbg_hsum", [H, BS], F32,
                                  kind="ExternalOutput")
        dbg_hy = nc.dram_tensor("dbg_hy", [64, BS], F32, kind="ExternalOutput")
        dbg_f2 = nc.dram_tensor("dbg_f2", [16, BS], F32, kind="ExternalOutput")

    with tile.TileContext(nc) as tc:
        with (
            tc.tile_pool(name="xp", bufs=1) as xp,
            tc.tile_pool(name="wp", bufs=1) as wp,
            tc.tile_pool(name="st", bufs=1) as st,
        ):
            # ---- weight / input DMAs ----
            # day-block split so conv can start before the full news lands
            DBLK = ((0, 3), (3, 5), (5, 8), (8, 10))
            # x8: e-pairs 0:256 as fp8 [ki, s, pos] for DoubleRow passes
            x8 = xp.tile([128, 2, D * BS * N], FP8, tag="x8")
            # xm01 interleaved: partition 2i = e-chunk2 row i, partition
            # 2i+1 = same row shifted one news position (dense K=88
            # tap-pair packing, full-width DMA via overlapping source AP)
            xm01 = xp.tile([88, D * BS * N], BF16, tag="xm01")
            dum = xp.tile([128, 512], BF16, tag="dum")
            nc.vector.memset(dum[:], 0.0)
            cw_sb = {k: wp.tile([128, 2, k, 112], FP8, tag=f"cw{k}",
                                name=f"cwsb{k}") for k in KS}
            cws_sb = wp.tile([88, 2, OC], BF16, tag="cws")
            cwm_sb = wp.tile([88, 5, OC], BF16, tag="cwm")
            wih_sb = wp.tile([101, 3, 4 * H], BF16, tag="wih")
            whh_sb = wp.tile([H, 4 * H], BF16, tag="whh")
            bias_sb = wp.tile([128, 71], F32, tag="bias")
            lxw_sb = wp.tile([128, 2, H], BF16, tag="lxw")
            lyw_sb = wp.tile([H, 64], BF16, tag="lyw")
            sft_sb = wp.tile([STATE_DIM, BS], BF16, tag="sft")
            w1t_sb = wp.tile([STATE_DIM, 24], BF16, tag="w1t")
            w2t_sb = wp.tile([24, 16], BF16, tag="w2t")
            aw1s_sb = wp.tile([64, MID], BF16, tag="aw1s")
            aw1f_sb = wp.tile([16, MID], BF16, tag="aw1f")

            # two fast HWDGE rings (sync / scalar), deadline-ordered;
            # gpsimd (slow SWDGE) only carries tiny head weights
            n2t = news2[:].tensor

            def news_block(bi):
                lo, hi = DBLK[bi]
                a, b = lo * BS * N, hi * BS * N
                nc.sync.dma_start(x8[:, :, a:b], news8[:, :, a:b])
                blk = b - a - (1 if hi == D else 0)
                m_src = bass.AP(tensor=n2t, offset=a,
                                ap=[[D * BS * N, 44], [1, 2], [1, blk]])
                nc.scalar.dma_start(xm01[0:88, a:a + blk], m_src)

            news_block(0)
            nc.sync.dma_start(cw_sb[3][:], cw8[3][:])
            nc.scalar.dma_start(cw_sb[4][:], cw8[4][:])
            nc.sync.dma_start(cw_sb[5][:], cw8[5][:])
            nc.scalar.dma_start(cwm_sb[:], cwm[:])
            nc.scalar.dma_start(cws_sb[:], cws[:])
            news_block(1)
            nc.scalar.dma_start(bias_sb[:], bias[:])
            nc.scalar.dma_start(sft_sb[:], sft[:])
            nc.scalar.dma_start(w1t_sb[:], w1t[:])
            nc.scalar.dma_start(w2t_sb[:], w2t[:])
            nc.sync.dma_start(wih_sb[:], wih[:])
            nc.scalar.dma_start(whh_sb[:], whh[:])
            news_block(2)
            news_block(3)
            nc.sync.dma_start(lxw_sb[:], lxw[:])
            nc.scalar.dma_start(lyw_sb[:], lyw[:])
            nc.sync.dma_start(aw1s_sb[:], aw1s[:])
            nc.scalar.dma_start(aw1f_sb[:], aw1f[:])

            # views of news: [p, (s,) day, batch, news-pos]
            x8v = x8[:].rearrange("p s (d b n) -> p s d b n", d=D, b=BS)
            xmv = xm01[:].rearrange("p (d b n) -> p d b n", d=D, b=BS)

            # persistent state
            textk = [st.tile([OC + 1 if c == 0 else OC, D * BS], BF16,
                             tag=f"text{c}", name=f"text{c}")
                     for c in range(3)]
            nc.vector.memset(textk[0][:], 1.0)
            h_sb = st.tile([H, BS], F32, tag="h")
            c_sb = st.tile([H, BS], F32, tag="c")
            hsum = st.tile([H, BS], F32, tag="hsum")
            nc.vector.memset(h_sb[:], 0.0)
            nc.vector.memset(c_sb[:], 0.0)
            tanc = st.tile([H, BS], F32, tag="tanc")
            tmp = st.tile([H, BS], F32, tag="tmp")
            h_bf = st.tile([H, BS], BF16, tag="h_bf")
            hsum_bf = st.tile([H, BS], BF16, tag="hsum_bf")
            nc.vector.memset(h_bf[:], 0.0)

            with (
                tc.tile_pool(name="psA", bufs=2, space="PSUM") as psA,
                tc.tile_pool(name="psPre", bufs=2, space="PSUM") as psPre,
                tc.tile_pool(name="gp", bufs=3) as gp,
            ):
                # PE warmup during the news DMA window
                for r in range(9):
                    wps = psA.tile([OC, 512], F32, tag="conv5",
                                   name=f"warm{r}")
                    nc.tensor.matmul(wps[:], dum[:, 0:OC], dum[:],
                                     start=True, stop=True)

                def conv_day(d):
                    for ki_, k in enumerate(KS):
                        L = N - k + 1
                        cp = psA.tile([112, BS, L], F32, tag=f"conv{k}",
                                      name=f"cp{k}_{d}")
                        # fp8 DoubleRow passes: 256 taps (e 0:256) per j
                        for j in range(k):
                            nc.tensor.matmul(
                                cp[:], cw_sb[k][:, :, j, :],
                                x8v[:, :, d, :, j:j + L],
                                start=(j == 0), stop=False,
                                skip_group_check=True, perf_mode=DROW)
                        # bf16 remainder (e 256:300): dense 88-row merged
                        # tap-pairs + leftover 44-row taps
                        passes = []
                        mslot = {3: [0], 4: [1, 2], 5: [3, 4]}[k]
                        for pi, mi in enumerate(mslot):
                            j = 2 * pi
                            passes.append((cwm_sb[:, mi, :],
                                           xmv[:, d, :, j:j + L]))
                        if k in (3, 5):
                            si_ = 0 if k == 3 else 1
                            j = k - 1
                            passes.append((cws_sb[:, si_, :],
                                           xmv[:, d, :, j:j + L]))
                        for i, (lh, rh) in enumerate(passes):
                            nc.tensor.matmul(cp[0:OC, :, :], lh, rh,
                                             start=False,
                                             stop=(i == len(passes) - 1),
                                             skip_group_check=True)
                        nc.vector.reduce_max(
                            textk[ki_][0:OC, d * BS:(d + 1) * BS],
                            cp[0:OC, :, :], axis=AX.X)

                def pre_pair(dp):
                    # NOTE: start=True clears has_written for the whole PSUM
                    # bank, so only the first matmul touching the bank sets
                    # it; later matmuls rely on per-element has_written bits.
                    pre_t = psPre.tile([128, 2, 4, BS], F32, tag="pre",
                                       name=f"pre{dp}")
                    for g in range(4):
                        for c in range(3):
                            kc = OC + 1 if c == 0 else OC
                            nc.tensor.matmul(
                                pre_t[:, :, g, :],
                                wih_sb[0:kc, c, g * H:(g + 1) * H],
                                textk[c][:, dp * 2 * BS:(dp + 1) * 2 * BS],
                                start=(g == 0 and c == 0), stop=False,
                                skip_group_check=True)
                    return pre_t

                def lstm_step(d, pre_t):
                    dsub = d % 2
                    for g in range(4):
                        nc.tensor.matmul(
                            pre_t[:, dsub, g, :],
                            whh_sb[:, g * H:(g + 1) * H],
                            h_bf[:],
                            start=False, stop=True,
                            skip_group_check=True)
                    # gate bias rides the K=101 ones-row; tanh(x) for the
                    # g-gate is computed as 2*sigmoid(2x)-1 (weights 2x'd on
                    # host) so one table lookup covers all four gates.
                    act = gp.tile([128, 4, BS], F32, tag="act",
                                  name=f"act{d}")
                    nc.scalar.activation(act[:], pre_t[:, dsub, :, :],
                                         AFT.Sigmoid)
                    i_s = act[:, 0, :]
                    f_s = act[:, 1, :]
                    o_s = act[:, 2, :]
                    g2 = gp.tile([128, BS], F32, tag="g2", name=f"g2_{d}")
                    nc.vector.tensor_scalar(g2[:], act[:, 3, :], 2.0, -1.0,
                                            AluOp.mult, AluOp.add)
                    nc.vector.tensor_mul(c_sb[:], c_sb[:], f_s)
                    nc.vector.tensor_mul(tmp[:], i_s, g2[:])
                    nc.vector.tensor_add(c_sb[:], c_sb[:], tmp[:])
                    nc.scalar.activation(tanc[:], c_sb[:], AFT.Tanh)
                    nc.vector.tensor_mul(h_sb[:], o_s, tanc[:])
                    nc.vector.tensor_mul(h_bf[:], o_s, tanc[:])
                    if d == 0:
                        nc.vector.tensor_copy(hsum[:], h_sb[:])
                    else:
                        nc.vector.tensor_add(hsum[:], hsum[:], h_sb[:])

                # software pipeline: gates lag conv by one day-pair,
                # interleaved between conv days so the PE never stalls on
                # the LSTM recurrence chain
                pre_tiles = {}
                for dp in range(5):
                    conv_day(2 * dp)
                    if dp == 1:
                        # feats branch (tiny, independent) rides day 2
                        fp1 = psA.tile([24, BS], F32, tag="conv4",
                                       name="fp1")
                        nc.tensor.matmul(fp1[:], w1t_sb[:], sft_sb[:],
                                         start=True, stop=True)
                        f1 = st.tile([24, BS], BF16, tag="f1")
                        nc.scalar.activation(f1[:], fp1[:], AFT.Relu,
                                             bias=bias_sb[0:24, 68:69])
                        fp2 = psA.tile([16, BS], F32, tag="conv4",
                                       name="fp2")
                        nc.tensor.matmul(fp2[:], w2t_sb[:], f1[:],
                                         start=True, stop=True)
                        f2 = st.tile([16, BS], BF16, tag="f2")
                        nc.scalar.activation(f2[:], fp2[:], AFT.Identity,
                                             bias=bias_sb[0:16, 69:70])
                    if dp > 0:
                        lstm_step(2 * dp - 2, pre_tiles[dp - 1])
                    conv_day(2 * dp + 1)
                    if dp > 0:
                        lstm_step(2 * dp - 1, pre_tiles[dp - 1])
                    pre_tiles[dp] = pre_pair(dp)
                lstm_step(8, pre_tiles[4])
                lstm_step(9, pre_tiles[4])

            # ---- head: linX, linY, feats branch, arch partial ----
            with (
                tc.tile_pool(name="ps2", bufs=1, space="PSUM") as ps2,
                tc.tile_pool(name="fin", bufs=1) as fin,
            ):
                nc.vector.tensor_copy(hsum_bf[:], hsum[:])
                px = ps2.tile([H, BS], F32, tag="px")
                nc.tensor.matmul(px[:], lxw_sb[:, 0, :], h_bf[:],
                                 start=True, stop=False)
                nc.tensor.matmul(px[:], lxw_sb[:, 1, :], hsum_bf[:],
                                 start=False, stop=True)
                hx = fin.tile([H, BS], BF16, tag="hx")
                nc.scalar.activation(hx[:], px[:], AFT.Relu,
                                     bias=bias_sb[:, 64:65])
                py = ps2.tile([64, BS], F32, tag="py")
                nc.tensor.matmul(py[:], lyw_sb[:], hx[:],
                                 start=True, stop=True)
                hy = fin.tile([64, BS], BF16, tag="hy")
                nc.scalar.activation(hy[:], py[:], AFT.Relu,
                                     bias=bias_sb[0:64, 65:66])

                qp = ps2.tile([128, 2, BS], F32, tag="qp")
                for m in range(2):
                    nc.tensor.matmul(qp[:, m, :],
                                     aw1s_sb[:, m * 128:(m + 1) * 128],
                                     hy[:], start=True, stop=False)
                    nc.tensor.matmul(qp[:, m, :],
                                     aw1f_sb[:, m * 128:(m + 1) * 128],
                                     f2[:], start=False, stop=True)
                qsb = fin.tile([128, 2, BS], F32, tag="qsb")
                for m in range(2):
                    nc.scalar.activation(qsb[:, m, :], qp[:, m, :],
                                         AFT.Identity,
                                         bias=bias_sb[:, 66 + m:67 + m])
                nc.sync.dma_start(outq[:], qsb[:])
                if debug_taps:
                    for c in range(3):
                        nc.sync.dma_start(dbg_text[c], textk[c][0:OC, :])
                    nc.sync.dma_start(dbg_h[:], h_sb[:])
                    nc.sync.dma_start(dbg_hsum[:], hsum[:])
                    nc.sync.dma_start(dbg_hy[:], hy[:])
                    nc.sync.dma_start(dbg_f2[:], f2[:])

    nc.compile()
    return nc


def _prep_inputs(inp):
    """Host-side shard/layout prep. Returns (in_maps, aw2, ab2, sigma)."""
    f32 = np.float32
    news = np.asarray(inp["stock_news"], f32)      # [B,S,D,N,E]
    # -> [S, E, D, B, N]
    newsT = np.ascontiguousarray(news.transpose(1, 4, 2, 0, 3))
    newsT = newsT.reshape(S, E, D * BS * N)

    sf = np.asarray(inp["stock_feats"], f32)
    w1 = np.asarray(inp["w1"], f32)
    w2 = np.asarray(inp["w2"], f32)
    aw1 = np.asarray(inp["arch_w1"], f32)          # [MID, 16+64*S]
    ab1 = np.asarray(inp["arch_b1"], f32)
    aw1f = np.ascontiguousarray((aw1[:, 64 * S:] / 8.0).T).astype(BF)
    sft = np.ascontiguousarray(sf.T).astype(BF)
    w1t = np.ascontiguousarray(w1.T).astype(BF)
    w2t = np.ascontiguousarray(w2.T).astype(BF)

    in_maps = []
    for s in range(S):
        m = {}
        # e 0:256 as fp8 [ki, s2, pos] (DoubleRow), e 256:300 as bf16
        m["news8"] = np.ascontiguousarray(
            newsT[s, 0:256]).reshape(128, 2, D * BS * N).astype(F8)
        m["news2"] = np.ascontiguousarray(newsT[s, 256:300]).astype(BF)
        cbs = []
        cws = np.zeros((88, 2, OC), np.float32)
        for k in KS:
            w = np.asarray(inp[f"conv_w{k}"], np.float32)[s]  # [OC, E, k]
            cbs.append(np.asarray(inp[f"conv_b{k}"], np.float32)[s])
            wt = w.transpose(1, 2, 0)                     # [E, k, OC]
            w8 = np.zeros((128, 2, k, 112), np.float32)
            w8[:, :, :, 0:OC] = wt[0:256].reshape(128, 2, k, OC)
            m[f"cw8{k}"] = w8.astype(F8)
            if k in (3, 5):
                cws[0::2, 0 if k == 3 else 1, :] = wt[256:300, k - 1, :]
        m["cws"] = cws.astype(BF)
        cwm = np.zeros((88, 5, OC), np.float32)
        for idx, (k, j) in enumerate(((3, 0), (4, 0), (4, 2), (5, 0), (5, 2))):
            w = np.asarray(inp[f"conv_w{k}"], np.float32)[s]  # [OC, E, k]
            cwm[0::2, idx, :] = w[:, 256:300, j].T
            cwm[1::2, idx, :] = w[:, 256:300, j + 1].T
        m["cwm"] = cwm.astype(BF)
        cb_full = np.concatenate(cbs)                     # [300]
        w_ih = np.asarray(inp["w_ih"], f32)[s]            # [4H, 300]
        w_hh = np.asarray(inp["w_hh"], f32)[s]            # [4H, H]
        b_ih = np.asarray(inp["b_ih"], f32)[s]
        b_hh = np.asarray(inp["b_hh"], f32)[s]
        # g-gate (cols 384:512 after perm) scaled 2x: tanh(x)=2*sig(2x)-1
        bias_eff = (b_ih + b_hh + w_ih @ cb_full)[_GPERM]  # [512]
        bias_eff[384:512] *= 2.0
        wihp = w_ih[_GPERM].T.copy()                      # [300, 512]
        wihp[:, 384:512] *= 2.0
        wih_arr = np.zeros((101, 3, 4 * H), f32)
        wih_arr[0:100] = wihp.reshape(3, 100, 4 * H).transpose(1, 0, 2)
        wih_arr[100, 0, :] = bias_eff                     # ones-row bias
        m["wih"] = np.ascontiguousarray(wih_arr).astype(BF)
        whhp = w_hh[_GPERM].T.copy()
        whhp[:, 384:512] *= 2.0
        m["whh"] = np.ascontiguousarray(whhp).astype(BF)

        bias = np.zeros((128, 71), f32)
        bias[:, 64] = np.asarray(inp["linX_b"], f32)[s]
        bias[0:64, 65] = np.asarray(inp["linY_b"], f32)[s]
        bias[:, 66:68] = (ab1 / 8.0).reshape(2, 128).T
        bias[0:24, 68] = np.asarray(inp["b1"], f32)
        bias[0:16, 69] = np.asarray(inp["b2"], f32)
        m["bias"] = bias

        lx = np.asarray(inp["linX_w"], f32)[s]            # [H, 2H]
        m["lxw"] = np.ascontiguousarray(
            lx.T.reshape(2, 128, H).transpose(1, 0, 2)).astype(BF)
        m["lyw"] = np.ascontiguousarray(
            np.asarray(inp["linY_w"], f32)[s].T).astype(BF)
        m["sft"] = sft
        m["w1t"] = w1t
        m["w2t"] = w2t
        m["aw1s"] = np.ascontiguousarray(aw1[:, 64 * s:64 * (s + 1)].T).astype(BF)
        m["aw1f"] = aw1f
        in_maps.append(m)

    aw2 = np.asarray(inp["arch_w2"], f32)                 # [S, MID]
    ab2 = np.asarray(inp["arch_b2"], f32)
    sigma = np.exp(np.asarray(inp["action_var"], f32))
    return in_maps, aw2, ab2, sigma


def run(inputs, trace=False, tmpdir=None, debug_taps=False):
    key = ("nc", debug_taps)
    if key not in _cache:
        _cache[key] = _build(debug_taps)
    nc = _cache[key]
    in_maps, aw2, ab2, sigma = _prep_inputs(inputs)
    res = run_bass_kernel_spmd(nc, in_maps, core_ids=list(range(NCORES)),
                               trace=trace, tmpdir=tmpdir)
    # host gather: sum partials -> relu -> layer 2 -> tanh
    A = np.zeros((MID, BS), np.float32)
    for c in range(NCORES):
        q = res.results[c]["outq"]                        # [128, 2, BS]
        A += q.transpose(1, 0, 2).reshape(MID, BS)
    a1 = np.maximum(A, 0.0)
    mu = np.tanh(aw2 @ a1 + ab2[:, None]).T               # [BS, S]
    return (np.ascontiguousarray(mu.astype(np.float32)),
            sigma.astype(np.float32)), res


def kernel(**inputs):
    (mu, sigma), _ = run(inputs, trace=False)
    return mu, sigma
